# revision 30
# baseline (speedup 1.0000x reference)
"""CRF loss (forward algorithm + gold score) on 8 trn2 NeuronCores.

Data-parallel over batch (32 sequences/core). The forward recurrence runs
in probability space with bf16 matmul operands (fp32 PSUM accumulation):
    v_t = (E^T @ v_{t-1}) * exp(e_t - c0),   E = exp(transitions)
Normalization is a constant per step (c0 ~ E[log colsum]); measured drift
stays within +-8 nats over 512 steps, far inside fp32 range.

To halve the serial chain depth, the kernel runs the forward (alpha) and
backward (beta) recursions CONCURRENTLY, meeting at t*=255:
    fwd:  v_t = (E^T @ v_{t-1}) ∘ F_t          (255 steps, weights E)
    bwd:  u_t = E @ (u_{t+1} ∘ F_{t+1})        (256 steps, weights E^T)
    logZ_b = log(sum_j v_255[j,b] * u_255[j,b]) + c_first + 511*c0
Each slot issues one fwd and one bwd step; the two chains pipeline across
the PE and DVE engines so wall time is ~256 serial steps, not 511.

Gold-path score runs on GPSIMD indirect_copy gathers, off the chain's
critical path:
  - emissions: each 16-partition group holds one sequence's emissions,
    split as 16 quarter-chunks (one per partition); the group's shared
    (wrapped) index list is (s % qsteps)*128 + tag[b,s]. A compile-time
    0/1 mask then selects, for gather s, the one partition whose chunk
    contains step s.
  - transitions: gather from a host-replicated [128, T*T] table with
    index tag_t*128 + tag_{t+1}.

Each core returns partial sums; the host combines them (unshard) into the
scalar loss.
"""

import numpy as np

import concourse.bacc as bacc
import concourse.mybir as mybir
import concourse.tile as tile
from concourse.bass_utils import run_bass_kernel_spmd
from concourse.mybir import AluOpType
from concourse.bass_isa import ReduceOp

F32 = mybir.dt.float32
BF16 = mybir.dt.bfloat16
I32 = mybir.dt.int32
U16 = mybir.dt.uint16

B, S, T = 256, 512, 128
NCORES = 8
BL = B // NCORES          # 32 sequences per core
CHUNK = 32                # F-tile chunk, in time steps

# log-domain normalization constants (E[log colsum] of this recurrence;
# anything within a few nats works — fp32 has ~88 nats of range)
C0 = 5.843
C_FIRST = 5.337

ACT_EXP = mybir.ActivationFunctionType.Exp
ACT_LN = mybir.ActivationFunctionType.Ln


def build_nc(s_steps=S, bl=BL, chunk=CHUNK):
    """Build the SPMD single-core program (identical on all cores)."""
    ngs = bl // 8             # indirect_copy gather sets (8 seqs each)
    qsteps = s_steps // 16    # steps per partition-quarter (emission gather)
    nw = s_steps // 16        # wrapped index-tile width
    tmid = s_steps // 2 - 1   # fwd/bwd meeting point t*

    nc = bacc.Bacc("TRN2", target_bir_lowering=False, debug=False,
                   enable_asserts=False)

    te2 = nc.dram_tensor("te2", [T, s_steps, bl], F32, kind="ExternalInput").ap()
    emq = nc.dram_tensor("emq", [ngs, 128, qsteps * T], F32,
                         kind="ExternalInput").ap()
    tagsw_a = nc.dram_tensor("tagsw_a", [bl // 8, 128, s_steps // 16], I32,
                             kind="ExternalInput").ap()
    tagsw_b = nc.dram_tensor("tagsw_b", [bl // 8, 128, s_steps // 16], I32,
                             kind="ExternalInput").ap()
    trans = nc.dram_tensor("trans", [T, T], F32, kind="ExternalInput").ap()
    transT = nc.dram_tensor("transT", [T, T], F32, kind="ExternalInput").ap()
    maskq = nc.dram_tensor("maskq", [128, s_steps], F32,
                           kind="ExternalInput").ap()
    cq = nc.dram_tensor("cq", [128, nw], I32, kind="ExternalInput").ap()
    out = nc.dram_tensor("out", [1, 8], F32, kind="ExternalOutput").ap()

    with tile.TileContext(nc) as tc:
        with (
            tc.tile_pool(name="const", bufs=1) as cpool,
            tc.tile_pool(name="stf", bufs=3) as stfpool,
            tc.tile_pool(name="stb", bufs=3) as stbpool,
            tc.tile_pool(name="ff", bufs=4) as ffpool,
            tc.tile_pool(name="fb", bufs=4) as fbpool,
            tc.tile_pool(name="vbuf", bufs=3) as vpool,
            tc.tile_pool(name="wbuf", bufs=3) as wpool,
            tc.tile_pool(name="psf", bufs=3, space="PSUM") as psfpool,
            tc.tile_pool(name="psb", bufs=3, space="PSUM") as psbpool,
            tc.tile_pool(name="pscs", bufs=1, space="PSUM") as cspool,
            tc.tile_pool(name="gold", bufs=1) as gpool,
            tc.tile_pool(name="eq", bufs=4) as eqpool,
            tc.tile_pool(name="gidx", bufs=2) as gipool,
            tc.tile_pool(name="gout", bufs=2) as gopool,
        ):
            bias_c0 = cpool.tile([128, 1], F32)
            nc.vector.memset(bias_c0[:], -C0)
            bias_cf = cpool.tile([128, 1], F32)
            nc.vector.memset(bias_cf[:], -C_FIRST)

            nchunk = s_steps // chunk

            def load_chunk(c, stpool, fpool):
                st = stpool.tile([T, chunk * bl], F32)
                nc.sync.dma_start(
                    st[:].rearrange("p (c b) -> p c b", c=chunk),
                    te2[:, c * chunk:(c + 1) * chunk, :])
                fc = fpool.tile([T, chunk * bl], F32)
                nc.scalar.activation(fc[:], st[:], ACT_EXP, bias=bias_c0[:])
                return st, fc

            # chain-init data first on the sync ring: the v0/w0 paths are
            # the longest poles to the first matmul
            st0, f_f = load_chunk(0, stfpool, ffpool)
            tr_raw = cpool.tile([T, T], F32)
            nc.sync.dma_start(tr_raw[:], trans)
            E = cpool.tile([T, T], BF16)
            nc.scalar.activation(E[:], tr_raw[:], ACT_EXP)
            stb, f_b = load_chunk(nchunk - 1, stbpool, fbpool)
            trT_raw = cpool.tile([T, T], F32, tag="trT")
            nc.sync.dma_start(trT_raw[:], transT)
            ET = cpool.tile([T, T], BF16)
            nc.scalar.activation(ET[:], trT_raw[:], ACT_EXP)

            # fwd init: v_0 = exp(e_0 - c_first)
            v = vpool.tile([T, bl], BF16)
            nc.scalar.activation(v[:], st0[:, 0:bl], ACT_EXP, bias=bias_cf[:])
            # bwd init: w_{S-1} = u_{S-1} ∘ F_{S-1} = F_{S-1}
            w = wpool.tile([T, bl], BF16)
            nc.scalar.activation(w[:], stb[:, (chunk - 1) * bl:chunk * bl],
                                 ACT_EXP, bias=bias_c0[:])

            # ---- gold score: emitted interleaved into the chain loop so
            # program-order priority spreads its DMAs/ops into engine gaps
            tr_rep = gpool.tile([128, T * T], F32)
            mask_t = gpool.tile([128, s_steps], F32)
            cq_t = gpool.tile([128, nw], I32)
            ecols = gpool.tile([128, ngs], F32)
            tcols = gpool.tile([128, ngs], F32)
            gold_a = []
            gold_eq = []
            gold_b = []

            def _prep():
                nc.gpsimd.dma_start(
                    tr_rep[0:1, :],
                    trans.rearrange("a b -> (a b)")[None, :])
                nc.gpsimd.dma_start(mask_t[:], maskq)
                nc.gpsimd.dma_start(cq_t[:], cq)
            gold_a.append(_prep)

            def _bcast():
                nc.gpsimd.partition_broadcast(tr_rep[:], tr_rep[0:1, :])
            gold_a.append(_bcast)

            def make_gset(g):
                tA = gipool.tile([128, nw], I32, tag="tA")
                tB = gipool.tile([128, nw], I32, tag="tB")

                eq_t = eqpool.tile([128, qsteps * T], F32)

                qf = qsteps * T // 4

                def _eq(q4):
                    def f():
                        nc.sync.dma_start(
                            eq_t[:, qf * q4:qf * (q4 + 1)],
                            emq[g, :, qf * q4:qf * (q4 + 1)])
                    return f
                for q4 in range(4):
                    gold_eq.append(_eq(q4))

                def _tags():
                    nc.gpsimd.dma_start(tA[:], tagsw_a[g])
                    nc.gpsimd.dma_start(tB[:], tagsw_b[g])
                gold_a.append(_tags)

                geo = gopool.tile([128, s_steps], F32, tag="geo")
                gto = gopool.tile([128, s_steps - 1], F32, tag="gto")

                def _gather():
                    eidx_f = gipool.tile([128, nw], F32, tag="eidxf")
                    nc.vector.tensor_tensor(eidx_f[:], tA[:], cq_t[:],
                                            AluOpType.add)
                    eidx = gipool.tile([128, nw], U16, tag="eidx")
                    nc.vector.tensor_copy(eidx[:], eidx_f[:])
                    tidx_f = gipool.tile([128, nw], F32, tag="tidxf")
                    nc.vector.scalar_tensor_tensor(
                        out=tidx_f[:], in0=tA[:], scalar=float(T), in1=tB[:],
                        op0=AluOpType.mult, op1=AluOpType.add)
                    tidx = gipool.tile([128, nw], U16, tag="tidx")
                    nc.vector.tensor_copy(tidx[:], tidx_f[:])
                    nc.gpsimd.indirect_copy(
                        geo[:], eq_t[:], eidx[:],
                        i_know_ap_gather_is_preferred=True)
                    nc.gpsimd.indirect_copy(
                        gto[:], tr_rep[:], tidx[:],
                        i_know_ap_gather_is_preferred=True)
                gold_b.append(_gather)

                def _reduce():
                    gem = gopool.tile([128, s_steps], F32, tag="gem")
                    nc.vector.tensor_tensor(gem[:], geo[:], mask_t[:],
                                            AluOpType.mult)
                    nc.vector.tensor_reduce(ecols[:, g:g + 1], gem[:],
                                            axis=mybir.AxisListType.X,
                                            op=AluOpType.add)
                    nc.vector.tensor_reduce(tcols[:, g:g + 1], gto[:],
                                            axis=mybir.AxisListType.X,
                                            op=AluOpType.add)
                gold_b.append(_reduce)

            for g in range(ngs):
                make_gset(g)

            cf_cur, cb_cur = 0, nchunk - 1
            it_a = iter(gold_a)
            nslots = s_steps // 2
            neq = len(gold_eq)
            eq_step = max(1, int(nslots * 0.72 / neq))
            eq_sched = {2 + i * eq_step: i for i in range(neq)}
            # consumer g fires a few slots after its last eq quarter lands
            b_sched = {}
            for g in range(ngs):
                slot = 2 + (4 * (g + 1) - 1) * eq_step + max(2, eq_step)
                slot = max(slot, 18 + g)
                b_sched[slot] = 2 * g
                b_sched[slot + 26] = 2 * g + 1
            b_done = set()
            eq_done = set()
            # ---- bidirectional chain: slot k advances fwd to t=k and bwd
            # to t=S-1-k ----
            for k in range(1, s_steps // 2 + 1):
                if k % 2 == 0 and k < 16:
                    th = next(it_a, None)
                    if th:
                        th()
                i_eq = eq_sched.get(k)
                if i_eq is not None and i_eq not in eq_done:
                    eq_done.add(i_eq)
                    gold_eq[i_eq]()
                g_b = b_sched.get(k)
                if g_b is not None and g_b not in b_done:
                    b_done.add(g_b)
                    gold_b[g_b]()
                tf = k                 # fwd step producing v_tf
                tb = s_steps - 1 - k   # bwd step producing u_tb
                if tf <= tmid:
                    c = tf // chunk
                    if c != cf_cur:
                        _, f_f = load_chunk(c, stfpool, ffpool)
                        cf_cur = c
                    pv = psfpool.tile([T, bl], F32)
                    nc.tensor.matmul(pv[:], lhsT=E[:], rhs=v[:],
                                     start=True, stop=True)
                    v = vpool.tile([T, bl], BF16)
                    kk = tf % chunk
                    nc.vector.tensor_tensor(v[:], pv[:],
                                            f_f[:, kk * bl:(kk + 1) * bl],
                                            AluOpType.mult)
                # bwd: u_tb = E @ w_{tb+1}
                pu = psbpool.tile([T, bl], F32)
                nc.tensor.matmul(pu[:], lhsT=ET[:], rhs=w[:],
                                 start=True, stop=True)
                if tb > tmid:
                    c = tb // chunk
                    if c != cb_cur:
                        _, f_b = load_chunk(c, stbpool, fbpool)
                        cb_cur = c
                    w = wpool.tile([T, bl], BF16)
                    kk = tb % chunk
                    nc.vector.tensor_tensor(w[:], pu[:],
                                            f_b[:, kk * bl:(kk + 1) * bl],
                                            AluOpType.mult)

            for th in it_a:
                th()
            for i_eq in range(neq):
                if i_eq not in eq_done:
                    gold_eq[i_eq]()
            for g_b in range(2 * ngs):
                if g_b not in b_done:
                    gold_b[g_b]()

            # ---- meet: logZ partial = sum_b log(sum_j v_t* u_t*) ----
            m = vpool.tile([T, bl], BF16, tag="meet")
            nc.vector.tensor_tensor(m[:], pu[:], v[:], AluOpType.mult)
            ones = cpool.tile([T, 1], BF16)
            nc.vector.memset(ones[:], 1.0)
            cs = cspool.tile([1, bl], F32)
            nc.tensor.matmul(cs[:], lhsT=ones[:], rhs=m[:],
                             start=True, stop=True)
            ln_t = gpool.tile([1, bl], F32)
            ln_acc = gpool.tile([1, 1], F32)
            nc.scalar.activation(ln_t[:], cs[:], ACT_LN, accum_out=ln_acc[:])

            ecol = gpool.tile([128, 1], F32)
            nc.vector.tensor_reduce(ecol[:], ecols[:],
                                    axis=mybir.AxisListType.X,
                                    op=AluOpType.add)
            eall = gpool.tile([128, 1], F32)
            nc.gpsimd.partition_all_reduce(eall[:], ecol[:], channels=128,
                                           reduce_op=ReduceOp.add)
            tcol = gpool.tile([128, 1], F32)
            nc.vector.tensor_reduce(tcol[:], tcols[:],
                                    axis=mybir.AxisListType.X,
                                    op=AluOpType.add)
            tall = gpool.tile([128, 1], F32)
            nc.gpsimd.partition_all_reduce(tall[:], tcol[:], channels=128,
                                           reduce_op=ReduceOp.add)

            # ---- assemble output ----
            osb = gpool.tile([1, 8], F32)
            nc.vector.memset(osb[:], 0.0)
            nc.vector.tensor_copy(osb[:, 0:1], ln_acc[:])
            nc.vector.tensor_copy(osb[:, 1:2], eall[0:1, :])
            nc.vector.tensor_copy(osb[:, 2:3], tall[0:1, :])
            nc.sync.dma_start(out, osb[:])

    nc.compile()
    return nc


_NC_CACHE = {}


def _get_nc(key=(S, BL, CHUNK)):
    if key not in _NC_CACHE:
        _NC_CACHE[key] = build_nc(*key)
    return _NC_CACHE[key]


def make_in_maps(emissions, tags, transitions, s_steps=S, bl=BL):
    """Shard full inputs into per-core input maps (host-side, layout only)."""
    emissions = np.ascontiguousarray(emissions, dtype=np.float32)
    transitions = np.ascontiguousarray(transitions, dtype=np.float32)
    tags = np.asarray(tags).astype(np.int32)
    ncores = emissions.shape[0] // bl
    ngs = bl // 8
    qsteps = s_steps // 16
    nw = s_steps // 16
    transT = np.ascontiguousarray(transitions.T)
    # mask[p, s] = 1 where partition p's quarter-chunk holds step s
    pp = np.arange(128) % 16
    ss = np.arange(s_steps) // qsteps
    maskq = np.ascontiguousarray(
        (pp[:, None] == ss[None, :]).astype(np.float32))
    # wrapped in-quarter offsets: index i=f*16+p' -> (i % qsteps) * T
    ii = (np.arange(nw)[None, :] * 16 + (np.arange(128) % 16)[:, None])
    cqv = np.ascontiguousarray(((ii % qsteps) * T).astype(np.int32))
    in_maps = []
    for c in range(ncores):
        em_c = emissions[c * bl:(c + 1) * bl, :s_steps]      # [bl, s, T]
        te2 = np.ascontiguousarray(em_c.transpose(2, 1, 0))  # [T, s, bl]
        emq = np.ascontiguousarray(em_c.reshape(ngs, 128, qsteps * T))
        tg = tags[c * bl:(c + 1) * bl, :s_steps]
        tgp = np.concatenate([tg, tg[:, -1:]], axis=1)  # [bl, s+1]
        # wrapped layouts: index i = f*16 + p' of seq b -> [g, 16j+p', f]
        wa = np.zeros((ngs, 128, nw), dtype=np.int32)
        wb = np.zeros((ngs, 128, nw), dtype=np.int32)
        for g in range(ngs):
            for j in range(8):
                b = 8 * g + j
                wa[g, 16 * j:16 * (j + 1), :] = (
                    tgp[b, :s_steps].reshape(nw, 16).T)
                wb[g, 16 * j:16 * (j + 1), :] = (
                    tgp[b, 1:s_steps + 1].reshape(nw, 16).T)
        in_maps.append({"te2": te2, "emq": emq,
                        "tagsw_a": np.ascontiguousarray(wa),
                        "tagsw_b": np.ascontiguousarray(wb),
                        "trans": transitions, "transT": transT,
                        "maskq": maskq, "cq": cqv})
    return in_maps


def combine(outs, s_steps=S, bl=BL):
    """Unshard: combine per-core partial sums into the scalar loss."""
    ln_sum = sum(float(o[0, 0]) for o in outs)
    emit_sum = sum(float(o[0, 1]) for o in outs)
    trans_sum = sum(float(o[0, 2]) for o in outs) / 16.0
    n = len(outs) * bl
    logz_mean = ln_sum / n + C_FIRST + (s_steps - 1) * C0
    gold_mean = (emit_sum + trans_sum) / n
    return np.float32(logz_mean - gold_mean)


def kernel(emissions, tags, transitions):
    nc = _get_nc()
    in_maps = make_in_maps(emissions, tags, transitions)
    res = run_bass_kernel_spmd(nc, in_maps, core_ids=list(range(NCORES)))
    outs = [r["out"] for r in res.results]
    return combine(outs)
